# revision 41
# baseline (speedup 1.0000x reference)
"""BitAttention TRN2 kernel: 8-core SPMD (DP over batch x TP over kv-heads).

Self-contained: hardcodes shapes B=2, S=2048, D=2048, H=16, KH=4.
Core r: batch b = r//4, kv-head kh = r%4, output token-quarter q# = r%4.

Math (forward-equivalent to the reference):
  - linear_bit = rms_norm -> per-row int8 act quant -> ternary weight quant ->
    matmul. Activations quantize to integers in [-127,127] (exact in bf16);
    ternary weights in {-1,0,1} (exact in bf16) -> projections run as
    exact-integer bf16 matmuls, dequant scales applied at PSUM eviction.
  - The reference einsum sums the query-head group axis, so Q's 16 heads
    collapse to 4 effective heads: group-sum the ternary w_q rows (ints in
    [-4,4], exact in bf16).
  - Both /sqrt(HD) scalings fold into one exact *(1/128) on q.
  - Ternary quant via threshold compare: round(tanh(w/(s+eps))) = sign(w) when
    |w| >= atanh(0.5)*(s+eps), else 0.
  - w_o is replicated on every core, so its |w| mean is computed locally and
    needs no AllReduce; w_o streams through SBUF twice (sum pass + ternary
    pass) during attention, off the critical path.
  - Scores are computed TRANSPOSED: S^T[key, query] = matmul(lhsT=kT, rhs=qT)
    with both already feature-major, so no P transposes are needed and the
    PSUM eviction fuses exp (softmax max-subtraction is skipped: scores are
    bounded by ||q||*||k|| <= ~2 here, far from exp overflow).
  - Softmax denominator Z rides along as a ones-column appended to V.
  - RoPE even/odd pairs are made contiguous by permuting w_q/w_k output dims
    (scores are invariant to a shared permutation of q/k feature dims).
Scheduling notes: engine queues are in-order, so the PE instruction stream is
software-pipelined (transposes of block i, then QKV matmuls of block i-1,
then rope-transposes of block i-2; in attention the S^T matmul of step n+1
precedes the PV matmuls of step n). All long-lived pools open before the
transient weight pools so no allocation waits on a pool-release barrier.
"""
import numpy as np
from contextlib import ExitStack

import concourse.bass as bass
import concourse.bacc as bacc
import concourse.mybir as mybir
import concourse.tile as tile
from concourse.bass_utils import run_bass_kernel_spmd
from concourse.masks import make_identity

B, S, D = 2, 2048, 2048
H, KH = 16, 4
HD = D // H          # 128
KVD = KH * HD        # 512
NB = S // 128        # 16 token blocks
SQ = S // 4          # 512 tokens per output quarter
EPS = 1e-8
MAGIC = float(1.5 * 2 ** 23)
ATANH05 = 0.5493061443340549      # arctanh(0.5)
NEGM = -60.0                      # causal mask additive value (pre-exp)
F32 = mybir.dt.float32
F16 = mybir.dt.float16
BF16 = mybir.dt.bfloat16
AX = mybir.AxisListType
OP = mybir.AluOpType
AF = mybir.ActivationFunctionType

_cache = {}


def build(causal: bool, local_cc: bool = False):
    nc = bacc.Bacc()
    x_d = nc.dram_tensor("x", [S, D], F32, kind="ExternalInput")
    wq_d = nc.dram_tensor("wq", [D, KVD], F32, kind="ExternalInput")   # sel+perm+T
    wk_d = nc.dram_tensor("wk", [D, HD], F32, kind="ExternalInput")    # perm+T
    wv_d = nc.dram_tensor("wv", [D, HD], F32, kind="ExternalInput")    # T
    wo_d = nc.dram_tensor("wo", [KVD, D], F32, kind="ExternalInput")   # w_o.T full
    cos_d = nc.dram_tensor("cos", [S, HD // 2], F32, kind="ExternalInput")
    sin_d = nc.dram_tensor("sin", [S, HD // 2], F32, kind="ExternalInput")
    y_d = nc.dram_tensor("y", [SQ, D], BF16, kind="ExternalOutput")
    st_in = nc.dram_tensor("st_in", [1, 4], F32)
    st_out = nc.dram_tensor("st_out", [1, 4], F32, addr_space="Shared")
    cc_in = nc.dram_tensor("cc_in", [4, SQ, HD], F32)
    cc_out = nc.dram_tensor("cc_out", [4, SQ, HD], F32)

    with tile.TileContext(nc) as tc, ExitStack() as ctx:
        cpool = ctx.enter_context(tc.tile_pool(name="const", bufs=1))
        sm = ctx.enter_context(tc.tile_pool(name="sm", bufs=1))
        wint = ctx.enter_context(tc.tile_pool(name="wint", bufs=1))
        xph = ctx.enter_context(tc.tile_pool(name="xph", bufs=1))
        qkv = ctx.enter_context(tc.tile_pool(name="qkv", bufs=1))
        attn = ctx.enter_context(tc.tile_pool(name="attn", bufs=1))
        outp = ctx.enter_context(tc.tile_pool(name="outp", bufs=1))
        psmm = ctx.enter_context(tc.tile_pool(name="psmm", bufs=2, space="PSUM"))
        pstp = ctx.enter_context(tc.tile_pool(name="pstp", bufs=2, space="PSUM"))
        pacc = ctx.enter_context(tc.tile_pool(name="pacc", bufs=1, space="PSUM"))

        # ---------- constants ----------
        idf = cpool.tile([128, 128], F32, tag="idf")
        make_identity(nc, idf[:])
        cmT = cpool.tile([128, 128], F32, tag="cmT")
        if causal:
            # transposed causal mask: keep (0) where key <= query, else NEGM
            nc.gpsimd.memset(cmT[:], 0.0)
            nc.gpsimd.affine_select(out=cmT[:], in_=cmT[:], compare_op=OP.is_ge,
                                    fill=NEGM, base=0, pattern=[[1, 128]],
                                    channel_multiplier=-1)
        ones_c = cpool.tile([128, 1], F32, tag="onc")
        nc.any.memset(ones_c[:], 1.0)
        ones_r = cpool.tile([1, 128], F32, tag="onr")
        nc.any.memset(ones_r[:], 1.0)
        inv_n = cpool.tile([128, 4], F32, tag="invn")
        for j, numel in enumerate([D * D, KVD * D, KVD * D, D * KVD]):
            nc.any.memset(inv_n[:, j:j + 1], 1.0 / (2.0 * numel))
        cos_all = cpool.tile([128, NB, HD // 2], F32, tag="cosall")
        sin_all = cpool.tile([128, NB, HD // 2], F32, tag="sinall")

        # persistent small tiles
        partials = sm.tile([128, 16], F32, tag="partials")
        ptot = sm.tile([128, 4], F32, tag="ptot")
        st_sb = sm.tile([1, 4], F32, tag="st_sb")
        st2_sb = sm.tile([1, 4], F32, tag="st2_sb")
        totals = sm.tile([128, 4], F32, tag="totals")
        s4 = sm.tile([128, 4], F32, tag="s4")
        thr4 = sm.tile([128, 4], F32, tag="thr4")
        nthr4 = sm.tile([128, 4], F32, tag="nthr4")
        a4 = sm.tile([128, 4], F32, tag="a4")
        aq128 = sm.tile([128, 1], F32, tag="aq128")
        ssq_all = sm.tile([128, NB], F32, tag="ssq_all")
        mx_all = sm.tile([128, NB], F32, tag="mx_all")
        deq_all = sm.tile([128, NB], F32, tag="deq_all")
        smul_all = sm.tile([128, NB], F32, tag="smul_all")
        dq_all = sm.tile([128, NB], F32, tag="dq_all")
        dk_all = sm.tile([128, NB], F32, tag="dk_all")
        dv_all = sm.tile([128, NB], F32, tag="dv_all")
        wop = sm.tile([128, 4], F32, tag="wop")
        so_t = sm.tile([128, 1], F32, tag="so_t")
        thr_o = sm.tile([128, 1], F32, tag="thr_o")
        nthr_o = sm.tile([128, 1], F32, tag="nthr_o")
        a_o = sm.tile([128, 1], F32, tag="a_o")
        so_row = sm.tile([1, 1], F32, tag="so_row")

        # persistent int weights / attention operands
        wqkv_i = [wint.tile([128, 3 * HD], BF16, tag=f"wi{j}", name=f"wi{j}")
                  for j in range(NB)]
        wo_i = [wint.tile([128, D], BF16, tag=f"wo{c}", name=f"wo{c}")
                for c in range(4)]
        qT = wint.tile([128, S], BF16, tag="qT")
        kT = wint.tile([128, S], BF16, tag="kT")
        v_aug = wint.tile([128, NB, HD + 4], BF16, tag="vaug")
        nc.any.memset(v_aug[:], 1.0)   # col HD stays 1.0 -> Z accumulator

        # ---------- weights wq/wk/wv: load, |w| sums, AllReduce, ternary -----
        with tc.tile_pool(name="wraw", bufs=1, side="right") as wraw:
            wqf = wraw.tile([128, NB, KVD], F32, tag="wqf")       # 4MB
            wkvf = wraw.tile([128, NB, 2 * HD], F32, tag="wkvf")  # 2MB
            nc.sync.dma_start(wkvf[:, :, 0:HD],
                              wk_d.ap().rearrange("(j p) f -> p j f", p=128))
            nc.sync.dma_start(wkvf[:, :, HD:2 * HD],
                              wv_d.ap().rearrange("(j p) f -> p j f", p=128))
            nc.sync.dma_start(wqf[:, 0:NB // 2, :],
                              wq_d.ap()[0:S // 2, :].rearrange("(j p) f -> p j f", p=128))
            nc.sync.dma_start(wqf[:, NB // 2:NB, :],
                              wq_d.ap()[S // 2:S, :].rearrange("(j p) f -> p j f", p=128))
            nc.vector.tensor_reduce(ptot[:, 1:2], wkvf[:, :, 0:HD],
                                    axis=AX.XY, op=OP.add, apply_absolute_value=True)
            nc.vector.tensor_reduce(ptot[:, 2:3], wkvf[:, :, HD:2 * HD],
                                    axis=AX.XY, op=OP.add, apply_absolute_value=True)
            nc.vector.tensor_reduce(ptot[:, 0:1], wqf[:, 0:NB // 2, :],
                                    axis=AX.XY, op=OP.add, apply_absolute_value=True)
            nc.vector.tensor_reduce(partials[:, 0:1], wqf[:, NB // 2:NB, :],
                                    axis=AX.XY, op=OP.add, apply_absolute_value=True)
            nc.vector.tensor_tensor(ptot[:, 0:1], ptot[:, 0:1], partials[:, 0:1],
                                    op=OP.add)
            nc.vector.memset(ptot[:, 3:4], 0.0)
            pcol = psmm.tile([1, 4], F32, tag="mm")
            nc.tensor.matmul(pcol[:], ones_c[:], ptot[:], start=True, stop=True)
            nc.vector.tensor_copy(st_sb[:], pcol[:])
            nc.gpsimd.dma_start(st_in[:], st_sb[:])
            if local_cc:
                nc.gpsimd.dma_start(st_out.ap(), st_in.ap())
            else:
                nc.gpsimd.collective_compute(
                    "AllReduce", OP.add, replica_groups=[list(range(8))],
                    ins=[st_in.ap().opt()], outs=[st_out.ap().opt()])
            nc.gpsimd.dma_start(st2_sb[:], st_out[:])
            bc = psmm.tile([128, 4], F32, tag="mm")
            nc.tensor.matmul(bc[:], ones_r[:], st2_sb[:], start=True, stop=True)
            nc.vector.tensor_copy(totals[:], bc[:])
            nc.vector.tensor_tensor(s4[:], totals[:], inv_n[:], op=OP.mult)
            nc.vector.tensor_scalar(thr4[:], s4[:], EPS, ATANH05, op0=OP.add, op1=OP.mult)
            nc.vector.tensor_scalar(nthr4[:], thr4[:], -1.0, None, op0=OP.mult)
            # a4 = 0.5*ln((1+s)/(1-s)) = arctanh(s)
            num = sm.tile([128, 4], F32, tag="num")
            den = sm.tile([128, 4], F32, tag="den")
            rat = sm.tile([128, 4], F32, tag="rat")
            nc.vector.tensor_scalar(num[:], s4[:], 1.0, None, op0=OP.add)
            nc.vector.tensor_scalar(den[:], s4[:], -1.0, 1.0, op0=OP.mult, op1=OP.add)
            nc.vector.reciprocal(rat[:], den[:])
            nc.vector.tensor_tensor(rat[:], rat[:], num[:], op=OP.mult)
            lnr = sm.tile([128, 4], F32, tag="lnr")
            nc.scalar.activation(lnr[:], rat[:], AF.Ln)
            nc.vector.tensor_scalar(a4[:], lnr[:], 0.5, None, op0=OP.mult)
            nc.vector.tensor_scalar(aq128[:], a4[:, 0:1], 1.0 / 128.0, None, op0=OP.mult)

            # ternary: tern(w) = (w >= thr) - (w < -thr), exact in bf16
            with tc.tile_pool(name="tern", bufs=2, side="right") as tp_:
                def ternary(src_ap, w, thr_t, nthr_t, out_ap, flip=0):
                    a_ = tp_.tile([128, w], BF16, tag="ta")
                    b_ = tp_.tile([128, w], BF16, tag="tb")
                    ea, eb = ((nc.vector, nc.gpsimd) if flip == 0
                              else (nc.gpsimd, nc.vector))
                    ea.tensor_scalar(a_[:, :w], src_ap, thr_t, None, op0=OP.is_ge)
                    eb.tensor_scalar(b_[:, :w], src_ap, nthr_t, -1.0,
                                     op0=OP.is_lt, op1=OP.mult)
                    nc.vector.tensor_tensor(out_ap, a_[:, :w], b_[:, :w], op=OP.add)

                for j in range(NB):
                    tq = tp_.tile([128, KVD], BF16, tag="tq")
                    ternary(wqf[:, j, :], KVD, thr4[:, 0:1], nthr4[:, 0:1],
                            tq[:, :], flip=j % 2)
                    e1 = tp_.tile([128, HD], BF16, tag="e1")
                    e2 = tp_.tile([128, HD], BF16, tag="e2")
                    nc.vector.tensor_tensor(e1[:], tq[:, 0:HD], tq[:, HD:2 * HD],
                                            op=OP.add)
                    nc.gpsimd.tensor_tensor(e2[:], tq[:, 2 * HD:3 * HD],
                                            tq[:, 3 * HD:4 * HD], op=OP.add)
                    nc.vector.tensor_tensor(wqkv_i[j][:, 0:HD], e1[:], e2[:], op=OP.add)
                    ternary(wkvf[:, j, 0:HD], HD, thr4[:, 1:2], nthr4[:, 1:2],
                            wqkv_i[j][:, HD:2 * HD], flip=j % 2)
                    ternary(wkvf[:, j, HD:2 * HD], HD, thr4[:, 2:3], nthr4[:, 2:3],
                            wqkv_i[j][:, 2 * HD:3 * HD], flip=(j + 1) % 2)

        # ---------- x phase fused with QKV (per-2-block waves) ----------
        sq_scr = xph.tile([128, D], BF16, tag="sqscr")
        hh = HD // 2
        xqTb_of = {}
        ropes_of = {}

        def emit_quant_T(i, xb):
            xm = xph.tile([128, D], F32, tag="xm", bufs=2)
            if i % 3 == 0:
                nc.scalar.activation(xm[:], xb[:], AF.Copy,
                                     scale=smul_all[:, i:i + 1], bias=MAGIC)
            else:
                nc.gpsimd.tensor_scalar(xm[:], xb[:],
                                        smul_all[:, i:i + 1], MAGIC,
                                        op0=OP.mult, op1=OP.add)
            xqTb = xph.tile([128, NB, 128], BF16, tag="xqTb", bufs=4)
            for jj in range(4):
                tp4 = pstp.tile([128, 512], F32, tag="tp")
                for t4 in range(4):
                    nc.tensor.transpose(
                        tp4[:, t4 * 128:(t4 + 1) * 128],
                        xm[:, (4 * jj + t4) * 128:(4 * jj + t4 + 1) * 128],
                        idf[:])
                dst = xqTb[:, 4 * jj:4 * jj + 4, :]
                if (i * 4 + jj) % 2 == 0:
                    nc.scalar.activation(dst, tp4[:], AF.Copy, bias=-MAGIC)
                else:
                    nc.vector.tensor_scalar(dst, tp4[:], -MAGIC, None, op0=OP.add)
            xqTb_of[i] = xqTb

        dq_done = set()

        def emit_qkvmm(i):
            wv0 = i // 2
            if wv0 not in dq_done:
                dq_done.add(wv0)
                sl = slice(wv0 * 2, wv0 * 2 + 2)
                nc.vector.tensor_scalar(dq_all[:, sl], deq_all[:, sl],
                                        aq128[:], None, op0=OP.mult)
                nc.vector.tensor_scalar(dk_all[:, sl], deq_all[:, sl],
                                        a4[:, 1:2], None, op0=OP.mult)
                nc.vector.tensor_scalar(dv_all[:, sl], deq_all[:, sl],
                                        a4[:, 2:3], None, op0=OP.mult)
            xqTb = xqTb_of.pop(i)
            pq = psmm.tile([128, 512], F32, tag="mm")
            for j in range(NB):
                nc.tensor.matmul(pq[:, 0:3 * HD], xqTb[:, j, :],
                                 wqkv_i[j][:], start=(j == 0), stop=(j == NB - 1))
            qn = qkv.tile([128, HD], F32, tag="qn", bufs=3)
            kn = qkv.tile([128, HD], F32, tag="kn", bufs=3)
            nc.vector.tensor_scalar(qn[:], pq[:, 0:HD],
                                    dq_all[:, i:i + 1], None, op0=OP.mult)
            nc.scalar.activation(kn[:], pq[:, HD:2 * HD], AF.Copy,
                                 scale=dk_all[:, i:i + 1])
            nc.scalar.activation(v_aug[:, i, 0:HD], pq[:, 2 * HD:3 * HD],
                                 AF.Copy, scale=dv_all[:, i:i + 1])
            ci = cos_all[:, i, :]
            si = sin_all[:, i, :]
            qr = qkv.tile([128, HD], F32, tag="qr", bufs=3)
            kr = qkv.tile([128, HD], F32, tag="kr", bufs=3)
            for src_, dst_, eng, tg in ((qn, qr, nc.gpsimd, "q"),
                                        (kn, kr, nc.gpsimd, "k")):
                t1 = qkv.tile([128, hh], F32, tag=f"rt1{tg}", bufs=2)
                t2 = qkv.tile([128, hh], F32, tag=f"rt2{tg}", bufs=2)
                eng.tensor_tensor(t1[:], src_[:, 0:hh], ci, op=OP.mult)
                eng.tensor_tensor(t2[:], src_[:, hh:HD], si, op=OP.mult)
                eng.tensor_tensor(dst_[:, 0:hh], t1[:], t2[:], op=OP.subtract)
                t3 = qkv.tile([128, hh], F32, tag=f"rt3{tg}", bufs=2)
                t4_ = qkv.tile([128, hh], F32, tag=f"rt4{tg}", bufs=2)
                eng.tensor_tensor(t3[:], src_[:, 0:hh], si, op=OP.mult)
                eng.tensor_tensor(t4_[:], src_[:, hh:HD], ci, op=OP.mult)
                eng.tensor_tensor(dst_[:, hh:HD], t3[:], t4_[:], op=OP.add)
            ropes_of[i] = (qr, kr)

        def emit_qkT(i):
            qr, kr = ropes_of.pop(i)
            tqk = pstp.tile([128, 512], F32, tag="tp")
            nc.tensor.transpose(tqk[:, 0:128], qr[:], idf[:])
            nc.tensor.transpose(tqk[:, 128:256], kr[:], idf[:])
            nc.scalar.activation(qT[:, i * 128:(i + 1) * 128], tqk[:, 0:128], AF.Copy)
            nc.vector.tensor_copy(kT[:, i * 128:(i + 1) * 128], tqk[:, 128:256])
            if causal and i % 4 == 3:
                emit_attn_group(i // 4)
            elif not causal and i == NB - 1:
                for qg_ in range(4):
                    emit_attn_group(qg_)

        def emit_wave(wv_, drain):
            i0 = wv_ * 2
            xw = []
            for u in range(2):
                i = i0 + u
                xb = xph.tile([128, D], F32, tag=f"xb{i % 3}", name="xb")
                xw.append(xb)
                nc.sync.dma_start(xb[:], x_d.ap()[i * 128:(i + 1) * 128, :])
                nc.scalar.activation(sq_scr[:], xb[:], AF.Square,
                                     accum_out=ssq_all[:, i:i + 1])
                nc.vector.tensor_reduce(mx_all[:, i:i + 1], xb[:],
                                        axis=AX.X, op=OP.max,
                                        apply_absolute_value=True)
            sl = slice(i0, i0 + 2)
            mean2 = xph.tile([128, 2], F32, tag="mean2", bufs=2)
            sd2 = xph.tile([128, 2], F32, tag="sd2", bufs=2)
            r2 = xph.tile([128, 2], F32, tag="r2", bufs=2)
            m2 = xph.tile([128, 2], F32, tag="m2", bufs=2)
            im2 = xph.tile([128, 2], F32, tag="im2", bufs=2)
            nc.vector.tensor_scalar(mean2[:], ssq_all[:, sl], 1.0 / D, EPS,
                                    op0=OP.mult, op1=OP.add)
            nc.scalar.activation(sd2[:], mean2[:], AF.Sqrt)
            nc.vector.reciprocal(r2[:], sd2[:])
            nc.vector.tensor_tensor(m2[:], r2[:], mx_all[:, sl], op=OP.mult)
            nc.vector.tensor_scalar(m2[:], m2[:], 1e-4, None, op0=OP.max)
            nc.vector.tensor_scalar(deq_all[:, sl], m2[:], 1.0 / 127.0,
                                    None, op0=OP.mult)
            nc.vector.reciprocal(im2[:], m2[:])
            nc.vector.tensor_scalar(im2[:], im2[:], 127.0, None, op0=OP.mult)
            nc.vector.tensor_tensor(smul_all[:, sl], r2[:], im2[:], op=OP.mult)
            for u in range(2):
                i = i0 + u
                emit_quant_T(i, xw[u])
                if drain:
                    while len(xqTb_of) > 1:
                        emit_qkvmm(min(xqTb_of))
                    while len(ropes_of) > 1:
                        emit_qkT(min(ropes_of))

        # ---------- attention machinery (emitted interleaved with X/QKV) -----
        # S^T[key,query] via matmul(lhsT=kT, rhs=qT); exp fused into PSUM
        # eviction; Z via ones-column in v_aug. Steps of query-group qg are
        # emitted as soon as qkT(4*qg+3) exists. PE stream pipelined 2 deep.
        from collections import deque
        accs_by_qg = {}
        pendq = deque()

        def emit_pv(qg, kb, pt, q0b):
            accs = accs_by_qg[qg]
            for i_ in range(q0b, 4 * qg + 4):
                t = i_ - 4 * qg
                off = (i_ - q0b) * 128
                done = (kb == i_) if causal else (kb == NB - 1)
                nc.tensor.matmul(accs[t], pt[:, off:off + 128],
                                 v_aug[:, kb, 0:HD + 1],
                                 start=(kb == 0), stop=done)
                if done:
                    rz = attn.tile([128, 1], F32, tag="rz", bufs=2)
                    nc.vector.reciprocal(rz[:], accs[t][:, HD:HD + 1])
                    ob = attn.tile([128, HD], BF16, tag="ob", bufs=2)
                    nc.scalar.activation(ob[:], accs[t][:, 0:HD],
                                         AF.Copy, scale=rz[:])
                    nc.sync.dma_start(cc_in[qg, t * 128:(t + 1) * 128, :], ob[:])

        def emit_attn_step(qg, kb):
            if kb == 0:
                accs_by_qg[qg] = [
                    pacc.tile([128, 132], F32, tag=f"acc{t}", name=f"acc{t}")[:, 0:HD + 1]
                    for t in range(4)]
            q0b = max(4 * qg, kb) if causal else 4 * qg
            w = (4 * qg + 4 - q0b) * 128
            ps = psmm.tile([128, 512], F32, tag="mm")
            nc.tensor.matmul(ps[:, 0:w], kT[:, kb * 128:(kb + 1) * 128],
                             qT[:, q0b * 128:q0b * 128 + w], start=True, stop=True)
            if causal and kb >= 4 * qg:
                nc.vector.tensor_tensor(ps[:, 0:128], ps[:, 0:128], cmT[:], op=OP.add)
            pt = attn.tile([128, 512], BF16, tag="pt", bufs=4)
            nc.scalar.activation(pt[:, 0:w], ps[:, 0:w], AF.Exp)
            pendq.append((qg, kb, pt, q0b))
            if len(pendq) > 1:
                emit_pv(*pendq.popleft())

        def emit_attn_group(qg):
            nkb = (4 * qg + 4) if causal else NB
            for kb in range(nkb):
                emit_attn_step(qg, kb)

        emit_wave(0, False)
        emit_wave(1, False)
        emit_ar_chain()
        emit_ternary()
        nc.sync.dma_start(cos_all[:], cos_d.ap().rearrange("(i p) f -> p i f", p=128))
        nc.sync.dma_start(sin_all[:], sin_d.ap().rearrange("(i p) f -> p i f", p=128))
        for wv_ in range(2, NB // 2):
            emit_wave(wv_, True)
        while xqTb_of:
            emit_qkvmm(min(xqTb_of))
        while ropes_of:
            emit_qkT(min(ropes_of))
        while pendq:
            emit_pv(*pendq.popleft())

        # ---------- w_o: one resident pass (local sum, then ternary) ---------
        # w_o is replicated on all cores -> local mean, no collective.
        with tc.tile_pool(name="wo2", bufs=1, side="right") as wo2:
            woss = []
            for c in range(4):
                wos = wo2.tile([128, D], F32, tag=f"wos{c}", name="wos")
                woss.append(wos)
                nc.sync.dma_start(wos[:], wo_d.ap()[c * 128:(c + 1) * 128, :])
                nc.vector.tensor_reduce(wop[:, c:c + 1], wos[:], axis=AX.X,
                                        op=OP.add, apply_absolute_value=True)
            wot = sm.tile([128, 1], F32, tag="wot")
            nc.vector.tensor_reduce(wot[:], wop[:], axis=AX.X, op=OP.add)
            pso = psmm.tile([1, 4], F32, tag="mm")
            nc.tensor.matmul(pso[:, 0:1], ones_c[:], wot[:], start=True, stop=True)
            nc.vector.tensor_copy(so_row[:], pso[:, 0:1])
            bco = psmm.tile([128, 4], F32, tag="mm")
            nc.tensor.matmul(bco[:, 0:1], ones_r[:], so_row[:], start=True, stop=True)
            nc.vector.tensor_scalar(so_t[:], bco[:, 0:1], 1.0 / (KVD * D), None,
                                    op0=OP.mult)
            nc.vector.tensor_scalar(thr_o[:], so_t[:], EPS, ATANH05,
                                    op0=OP.add, op1=OP.mult)
            nc.vector.tensor_scalar(nthr_o[:], thr_o[:], -1.0, None, op0=OP.mult)
            numo = sm.tile([128, 1], F32, tag="numo")
            deno = sm.tile([128, 1], F32, tag="deno")
            rato = sm.tile([128, 1], F32, tag="rato")
            nc.vector.tensor_scalar(numo[:], so_t[:], 1.0, None, op0=OP.add)
            nc.vector.tensor_scalar(deno[:], so_t[:], -1.0, 1.0, op0=OP.mult, op1=OP.add)
            nc.vector.reciprocal(rato[:], deno[:])
            nc.vector.tensor_tensor(rato[:], rato[:], numo[:], op=OP.mult)
            lno = sm.tile([128, 1], F32, tag="lno")
            nc.scalar.activation(lno[:], rato[:], AF.Ln)
            nc.vector.tensor_scalar(a_o[:], lno[:], 0.5, None, op0=OP.mult)
            for c in range(4):
                for hf in range(2):
                    slc = slice(hf * 1024, (hf + 1) * 1024)
                    a_ = wo2.tile([128, 1024], BF16, tag="toa", bufs=2)
                    b_ = wo2.tile([128, 1024], BF16, tag="tob", bufs=2)
                    nc.vector.tensor_scalar(a_[:], woss[c][:, slc], thr_o[:], None,
                                            op0=OP.is_ge)
                    nc.vector.tensor_scalar(b_[:], woss[c][:, slc], nthr_o[:], -1.0,
                                            op0=OP.is_lt, op1=OP.mult)
                    nc.vector.tensor_tensor(wo_i[c][:, slc], a_[:], b_[:], op=OP.add)

        # ---------- exchange: quad AllToAll (within batch group) -------------
        if local_cc:
            nc.sync.dma_start(cc_out.ap(), cc_in.ap())
        else:
            nc.gpsimd.collective_compute(
                "AllToAll", OP.bypass,
                replica_groups=[[0, 1, 2, 3], [4, 5, 6, 7]],
                ins=[cc_in.ap().opt()], outs=[cc_out.ap().opt()])

        # ---------- output projection ----------
        xoT = outp.tile([128, 4, SQ], BF16, tag="xoT")
        osc = outp.tile([128, KVD], BF16, tag="osc")
        xo4s = [outp.tile([128, 4, HD], BF16, tag=f"xo{tb}", name=f"xo{tb}")
                for tb in range(4)]
        sso = outp.tile([128, 4], F32, tag="sso")
        mxo = outp.tile([128, 4], F32, tag="mxo")
        for tb in range(4):
            nc.sync.dma_start(
                xo4s[tb][:],
                cc_out.ap()[:, tb * 128:(tb + 1) * 128, :].rearrange("j p d -> p j d"))
            nc.scalar.activation(osc[:], xo4s[tb][:], AF.Square,
                                 accum_out=sso[:, tb:tb + 1])
            nc.vector.tensor_reduce(mxo[:, tb:tb + 1], xo4s[tb][:],
                                    axis=AX.XY, op=OP.max, apply_absolute_value=True)
        meano = outp.tile([128, 4], F32, tag="meano")
        sdo = outp.tile([128, 4], F32, tag="sdo")
        ro = outp.tile([128, 4], F32, tag="ro")
        mo = outp.tile([128, 4], F32, tag="mo")
        imo = outp.tile([128, 4], F32, tag="imo")
        smul_o = outp.tile([128, 4], F32, tag="smul_o")
        dqy_o = outp.tile([128, 4], F32, tag="dqy_o")
        nc.vector.tensor_scalar(meano[:], sso[:], 1.0 / KVD, EPS, op0=OP.mult, op1=OP.add)
        nc.scalar.activation(sdo[:], meano[:], AF.Sqrt)
        nc.vector.reciprocal(ro[:], sdo[:])
        nc.vector.tensor_tensor(mo[:], ro[:], mxo[:], op=OP.mult)
        nc.vector.tensor_scalar(mo[:], mo[:], 1e-4, None, op0=OP.max)
        nc.vector.tensor_scalar(dqy_o[:], mo[:], 1.0 / 127.0, None, op0=OP.mult)
        nc.vector.tensor_scalar(dqy_o[:], dqy_o[:], a_o[:], None, op0=OP.mult)
        nc.vector.reciprocal(imo[:], mo[:])
        nc.vector.tensor_scalar(imo[:], imo[:], 127.0, None, op0=OP.mult)
        nc.vector.tensor_tensor(smul_o[:], ro[:], imo[:], op=OP.mult)

        def emit_oT(tb):
            xmo = outp.tile([128, KVD], F32, tag="xmo", bufs=2)
            nc.scalar.activation(xmo[:], xo4s[tb][:], AF.Copy,
                                 scale=smul_o[:, tb:tb + 1], bias=MAGIC)
            tpo = pstp.tile([128, 512], F32, tag="tp")
            for jc in range(4):
                nc.tensor.transpose(tpo[:, jc * 128:(jc + 1) * 128],
                                    xmo[:, jc * 128:(jc + 1) * 128], idf[:])
            dst = xoT[:, 0:4, tb * 128:(tb + 1) * 128]
            if tb % 2 == 0:
                nc.vector.tensor_scalar(dst, tpo[:], -MAGIC, None, op0=OP.add)
            else:
                nc.scalar.activation(dst, tpo[:], AF.Copy, bias=-MAGIC)

        def emit_omm(tb):
            y_sb = outp.tile([128, D], BF16, tag="ysb", bufs=2)
            for oc in range(4):
                py = psmm.tile([128, 512], F32, tag="mm")
                for jc in range(4):
                    nc.tensor.matmul(py[:], xoT[:, jc, tb * 128:(tb + 1) * 128],
                                     wo_i[jc][:, oc * 512:(oc + 1) * 512],
                                     start=(jc == 0), stop=(jc == 3))
                nc.scalar.activation(y_sb[:, oc * 512:(oc + 1) * 512], py[:],
                                     AF.Copy, scale=dqy_o[:, tb:tb + 1])
            nc.sync.dma_start(y_d[tb * 128:(tb + 1) * 128, :], y_sb[:])

        for tb in range(4):
            emit_oT(tb)
            if tb >= 1:
                emit_omm(tb - 1)
        emit_omm(3)
    nc.compile()
    return nc


def _rope_perm():
    p = np.empty(HD, np.int64)
    p[:HD // 2] = np.arange(0, HD, 2)
    p[HD // 2:] = np.arange(1, HD, 2)
    return p


def _prep_inputs(inputs):
    x = np.ascontiguousarray(np.asarray(inputs["x"], np.float32))
    w_q = np.asarray(inputs["w_q"], np.float32)
    w_k = np.asarray(inputs["w_k"], np.float32)
    w_v = np.asarray(inputs["w_v"], np.float32)
    w_o = np.asarray(inputs["w_o"], np.float32)
    cos = np.ascontiguousarray(np.asarray(inputs["freq_cos"], np.float32))
    sin = np.ascontiguousarray(np.asarray(inputs["freq_sin"], np.float32))
    perm = _rope_perm()
    woT = np.ascontiguousarray(w_o.T)                      # [KVD, D]
    in_maps = []
    for r in range(8):
        b, kh = r // 4, r % 4
        heads = [g * KH + kh for g in range(4)]
        wq_sel = w_q.reshape(H, HD, D)[heads][:, perm, :]  # [4,128,D]
        wqT = np.ascontiguousarray(wq_sel.reshape(4 * HD, D).T)   # [D, 512]
        wkT = np.ascontiguousarray(w_k[kh * HD:(kh + 1) * HD][perm].T)  # [D,128]
        wvT = np.ascontiguousarray(w_v[kh * HD:(kh + 1) * HD].T)        # [D,128]
        in_maps.append({
            "x": x[b], "wq": wqT, "wk": wkT, "wv": wvT, "wo": woT,
            "cos": cos, "sin": sin,
        })
    return in_maps


def _gains_trivial(inputs):
    return all(np.all(np.asarray(inputs[g]) == 1.0)
               for g in ("g_q", "g_k", "g_v", "g_o"))


def _numpy_fallback(inputs):
    """Faithful numpy reimplementation (slow); used only for unexpected configs."""
    x = np.asarray(inputs["x"], np.float32)
    cos, sin = (np.asarray(inputs[k], np.float32) for k in ("freq_cos", "freq_sin"))
    causal = int(np.asarray(inputs["causal"]))

    def rms(t, g):
        n = t * (1.0 / np.sqrt(np.mean(t * t, -1, keepdims=True, dtype=np.float32) + EPS))
        return (g * n).astype(np.float32)

    def actq(t):
        scale = 127.0 / np.clip(np.max(np.abs(t), -1, keepdims=True), 1e-4, None)
        q = np.round(t * scale)
        return np.clip(q, -128, 127) / scale

    def ternq(w):
        s = np.mean(np.abs(w), dtype=np.float32)
        return np.round(np.tanh(w / (s + EPS))) * np.arctanh(s)

    def lin(t, w, g):
        return actq(rms(t, g)).astype(np.float32) @ ternq(np.asarray(w, np.float32)).T

    Bb, Ss, Dd = x.shape
    q = lin(x, inputs["w_q"], np.asarray(inputs["g_q"], np.float32)).reshape(Bb, Ss, H, HD)
    k = lin(x, inputs["w_k"], np.asarray(inputs["g_k"], np.float32)).reshape(Bb, Ss, KH, HD)
    v = lin(x, inputs["w_v"], np.asarray(inputs["g_v"], np.float32)).reshape(Bb, Ss, KH, HD)

    def rope(t):
        t2 = t.reshape(*t.shape[:-1], -1, 2)
        c = cos[None, :, None, :]
        s_ = sin[None, :, None, :]
        o0 = t2[..., 0] * c - t2[..., 1] * s_
        o1 = t2[..., 0] * s_ + t2[..., 1] * c
        return np.stack([o0, o1], -1).reshape(t.shape).astype(np.float32)

    q, k = rope(q), rope(k)
    scale = np.float32(HD ** 0.5)
    q = q.transpose(0, 2, 1, 3) / scale
    k = k.transpose(0, 2, 1, 3)
    v = v.transpose(0, 2, 1, 3)
    qg = q.reshape(Bb, 4, KH, Ss, HD).sum(1)
    sc = np.einsum("bhnd,bhsd->bhns", qg, k).astype(np.float32)
    if causal:
        mask = np.tril(np.ones((Ss, Ss), bool))
        sc = np.where(mask[None, None], sc, np.float32(np.finfo(np.float32).min))
    sc = sc / scale
    sc = sc - sc.max(-1, keepdims=True)
    p = np.exp(sc)
    p /= p.sum(-1, keepdims=True)
    out = np.einsum("bhns,bhsd->bnhd", p, v).reshape(Bb, Ss, KVD)
    return lin(out, inputs["w_o"], np.asarray(inputs["g_o"], np.float32))


def kernel(**inputs):
    x = np.asarray(inputs["x"])
    if x.shape != (B, S, D) or not _gains_trivial(inputs):
        return _numpy_fallback(inputs)
    causal = bool(int(np.asarray(inputs["causal"])))
    key = ("bitattn", causal)
    if key not in _cache:
        _cache[key] = build(causal)
    nc = _cache[key]
    in_maps = _prep_inputs(inputs)
    res = run_bass_kernel_spmd(nc, in_maps, core_ids=list(range(8)))
    y = np.empty((B, S, D), np.float32)
    for r in range(8):
        b, qq = r // 4, r % 4
        y[b, qq * SQ:(qq + 1) * SQ, :] = np.asarray(res.results[r]["y"],
                                                    dtype=np.float32)
    return y


if __name__ == "__main__":
    data = np.load("/tmp/inputs.npz")
    inputs = {k: data[k] for k in data.files}
    out = kernel(**inputs)
    exp = np.load("/tmp/expected.npy")
    err = np.linalg.norm(out - exp) / np.linalg.norm(exp)
    print("Relative error:", err)


# revision 42
# speedup vs baseline: 1.0118x; 1.0118x over previous
"""BitAttention TRN2 kernel: 8-core SPMD (DP over batch x TP over kv-heads).

Self-contained: hardcodes shapes B=2, S=2048, D=2048, H=16, KH=4.
Core r: batch b = r//4, kv-head kh = r%4, output token-quarter q# = r%4.

Math (forward-equivalent to the reference):
  - linear_bit = rms_norm -> per-row int8 act quant -> ternary weight quant ->
    matmul. Activations quantize to integers in [-127,127] (exact in bf16);
    ternary weights in {-1,0,1} (exact in bf16) -> projections run as
    exact-integer bf16 matmuls, dequant scales applied at PSUM eviction.
  - The reference einsum sums the query-head group axis, so Q's 16 heads
    collapse to 4 effective heads: group-sum the ternary w_q rows (ints in
    [-4,4], exact in bf16).
  - Both /sqrt(HD) scalings fold into one exact *(1/128) on q.
  - Ternary quant via threshold compare: round(tanh(w/(s+eps))) = sign(w) when
    |w| >= atanh(0.5)*(s+eps), else 0.
  - w_o is replicated on every core, so its |w| mean is computed locally and
    needs no AllReduce; w_o streams through SBUF twice (sum pass + ternary
    pass) during attention, off the critical path.
  - Scores are computed TRANSPOSED: S^T[key, query] = matmul(lhsT=kT, rhs=qT)
    with both already feature-major, so no P transposes are needed and the
    PSUM eviction fuses exp (softmax max-subtraction is skipped: scores are
    bounded by ||q||*||k|| <= ~2 here, far from exp overflow).
  - Softmax denominator Z rides along as a ones-column appended to V.
  - RoPE even/odd pairs are made contiguous by permuting w_q/w_k output dims
    (scores are invariant to a shared permutation of q/k feature dims).
Scheduling notes: engine queues are in-order, so the PE instruction stream is
software-pipelined (transposes of block i, then QKV matmuls of block i-1,
then rope-transposes of block i-2; in attention the S^T matmul of step n+1
precedes the PV matmuls of step n). All long-lived pools open before the
transient weight pools so no allocation waits on a pool-release barrier.
"""
import numpy as np
from contextlib import ExitStack

import concourse.bass as bass
import concourse.bacc as bacc
import concourse.mybir as mybir
import concourse.tile as tile
from concourse.bass_utils import run_bass_kernel_spmd
from concourse.masks import make_identity

B, S, D = 2, 2048, 2048
H, KH = 16, 4
HD = D // H          # 128
KVD = KH * HD        # 512
NB = S // 128        # 16 token blocks
SQ = S // 4          # 512 tokens per output quarter
EPS = 1e-8
MAGIC = float(1.5 * 2 ** 23)
ATANH05 = 0.5493061443340549      # arctanh(0.5)
NEGM = -60.0                      # causal mask additive value (pre-exp)
F32 = mybir.dt.float32
F16 = mybir.dt.float16
BF16 = mybir.dt.bfloat16
AX = mybir.AxisListType
OP = mybir.AluOpType
AF = mybir.ActivationFunctionType

_cache = {}


def build(causal: bool, local_cc: bool = False):
    nc = bacc.Bacc()
    x_d = nc.dram_tensor("x", [S, D], F32, kind="ExternalInput")
    wq_d = nc.dram_tensor("wq", [D, KVD], F32, kind="ExternalInput")   # sel+perm+T
    wk_d = nc.dram_tensor("wk", [D, HD], F32, kind="ExternalInput")    # perm+T
    wv_d = nc.dram_tensor("wv", [D, HD], F32, kind="ExternalInput")    # T
    wo_d = nc.dram_tensor("wo", [KVD, D], F32, kind="ExternalInput")   # w_o.T full
    cos_d = nc.dram_tensor("cos", [S, HD // 2], F32, kind="ExternalInput")
    sin_d = nc.dram_tensor("sin", [S, HD // 2], F32, kind="ExternalInput")
    y_d = nc.dram_tensor("y", [SQ, D], BF16, kind="ExternalOutput")
    st_in = nc.dram_tensor("st_in", [1, 4], F32)
    st_out = nc.dram_tensor("st_out", [1, 4], F32, addr_space="Shared")
    cc_in = nc.dram_tensor("cc_in", [4, SQ, HD], F32)
    cc_out = nc.dram_tensor("cc_out", [4, SQ, HD], F32)

    with tile.TileContext(nc) as tc, ExitStack() as ctx:
        cpool = ctx.enter_context(tc.tile_pool(name="const", bufs=1))
        sm = ctx.enter_context(tc.tile_pool(name="sm", bufs=1))
        wint = ctx.enter_context(tc.tile_pool(name="wint", bufs=1))
        xph = ctx.enter_context(tc.tile_pool(name="xph", bufs=1))
        qkv = ctx.enter_context(tc.tile_pool(name="qkv", bufs=1))
        attn = ctx.enter_context(tc.tile_pool(name="attn", bufs=1))
        outp = ctx.enter_context(tc.tile_pool(name="outp", bufs=1))
        psmm = ctx.enter_context(tc.tile_pool(name="psmm", bufs=2, space="PSUM"))
        pstp = ctx.enter_context(tc.tile_pool(name="pstp", bufs=2, space="PSUM"))
        pacc = ctx.enter_context(tc.tile_pool(name="pacc", bufs=1, space="PSUM"))

        # ---------- constants ----------
        idf = cpool.tile([128, 128], F32, tag="idf")
        make_identity(nc, idf[:])
        cmT = cpool.tile([128, 128], F32, tag="cmT")
        if causal:
            # transposed causal mask: keep (0) where key <= query, else NEGM
            nc.gpsimd.memset(cmT[:], 0.0)
            nc.gpsimd.affine_select(out=cmT[:], in_=cmT[:], compare_op=OP.is_ge,
                                    fill=NEGM, base=0, pattern=[[1, 128]],
                                    channel_multiplier=-1)
        ones_c = cpool.tile([128, 1], F32, tag="onc")
        nc.any.memset(ones_c[:], 1.0)
        ones_r = cpool.tile([1, 128], F32, tag="onr")
        nc.any.memset(ones_r[:], 1.0)
        inv_n = cpool.tile([128, 4], F32, tag="invn")
        for j, numel in enumerate([D * D, KVD * D, KVD * D, D * KVD]):
            nc.any.memset(inv_n[:, j:j + 1], 1.0 / (2.0 * numel))
        cos_all = cpool.tile([128, NB, HD // 2], F32, tag="cosall")
        sin_all = cpool.tile([128, NB, HD // 2], F32, tag="sinall")

        # persistent small tiles
        partials = sm.tile([128, 16], F32, tag="partials")
        ptot = sm.tile([128, 4], F32, tag="ptot")
        st_sb = sm.tile([1, 4], F32, tag="st_sb")
        st2_sb = sm.tile([1, 4], F32, tag="st2_sb")
        totals = sm.tile([128, 4], F32, tag="totals")
        s4 = sm.tile([128, 4], F32, tag="s4")
        thr4 = sm.tile([128, 4], F32, tag="thr4")
        nthr4 = sm.tile([128, 4], F32, tag="nthr4")
        a4 = sm.tile([128, 4], F32, tag="a4")
        aq128 = sm.tile([128, 1], F32, tag="aq128")
        ssq_all = sm.tile([128, NB], F32, tag="ssq_all")
        mx_all = sm.tile([128, NB], F32, tag="mx_all")
        deq_all = sm.tile([128, NB], F32, tag="deq_all")
        smul_all = sm.tile([128, NB], F32, tag="smul_all")
        dq_all = sm.tile([128, NB], F32, tag="dq_all")
        dk_all = sm.tile([128, NB], F32, tag="dk_all")
        dv_all = sm.tile([128, NB], F32, tag="dv_all")
        wop = sm.tile([128, 4], F32, tag="wop")
        so_t = sm.tile([128, 1], F32, tag="so_t")
        thr_o = sm.tile([128, 1], F32, tag="thr_o")
        nthr_o = sm.tile([128, 1], F32, tag="nthr_o")
        a_o = sm.tile([128, 1], F32, tag="a_o")
        so_row = sm.tile([1, 1], F32, tag="so_row")

        # persistent int weights / attention operands
        wqkv_i = [wint.tile([128, 3 * HD], BF16, tag=f"wi{j}", name=f"wi{j}")
                  for j in range(NB)]
        wo_i = [wint.tile([128, D], BF16, tag=f"wo{c}", name=f"wo{c}")
                for c in range(4)]
        qT = wint.tile([128, S], BF16, tag="qT")
        kT = wint.tile([128, S], BF16, tag="kT")
        v_aug = wint.tile([128, NB, HD + 4], BF16, tag="vaug")
        nc.any.memset(v_aug[:], 1.0)   # col HD stays 1.0 -> Z accumulator

        # ---------- weights wq/wk/wv: load, |w| sums, AllReduce, ternary -----
        with tc.tile_pool(name="wraw", bufs=1, side="right") as wraw:
            wqf = wraw.tile([128, NB, KVD], F32, tag="wqf")       # 4MB
            wkvf = wraw.tile([128, NB, 2 * HD], F32, tag="wkvf")  # 2MB
            nc.sync.dma_start(wkvf[:, :, 0:HD],
                              wk_d.ap().rearrange("(j p) f -> p j f", p=128))
            nc.sync.dma_start(wkvf[:, :, HD:2 * HD],
                              wv_d.ap().rearrange("(j p) f -> p j f", p=128))
            nc.sync.dma_start(wqf[:, 0:NB // 2, :],
                              wq_d.ap()[0:S // 2, :].rearrange("(j p) f -> p j f", p=128))
            nc.sync.dma_start(wqf[:, NB // 2:NB, :],
                              wq_d.ap()[S // 2:S, :].rearrange("(j p) f -> p j f", p=128))
            nc.vector.tensor_reduce(ptot[:, 1:2], wkvf[:, :, 0:HD],
                                    axis=AX.XY, op=OP.add, apply_absolute_value=True)
            nc.vector.tensor_reduce(ptot[:, 2:3], wkvf[:, :, HD:2 * HD],
                                    axis=AX.XY, op=OP.add, apply_absolute_value=True)
            nc.vector.tensor_reduce(ptot[:, 0:1], wqf[:, 0:NB // 2, :],
                                    axis=AX.XY, op=OP.add, apply_absolute_value=True)
            nc.vector.tensor_reduce(partials[:, 0:1], wqf[:, NB // 2:NB, :],
                                    axis=AX.XY, op=OP.add, apply_absolute_value=True)
            nc.vector.tensor_tensor(ptot[:, 0:1], ptot[:, 0:1], partials[:, 0:1],
                                    op=OP.add)
            nc.vector.memset(ptot[:, 3:4], 0.0)
            pcol = psmm.tile([1, 4], F32, tag="mm")
            nc.tensor.matmul(pcol[:], ones_c[:], ptot[:], start=True, stop=True)
            nc.vector.tensor_copy(st_sb[:], pcol[:])
            nc.gpsimd.dma_start(st_in[:], st_sb[:])
            if local_cc:
                nc.gpsimd.dma_start(st_out.ap(), st_in.ap())
            else:
                nc.gpsimd.collective_compute(
                    "AllReduce", OP.add, replica_groups=[list(range(8))],
                    ins=[st_in.ap().opt()], outs=[st_out.ap().opt()])
            nc.gpsimd.dma_start(st2_sb[:], st_out[:])
            bc = psmm.tile([128, 4], F32, tag="mm")
            nc.tensor.matmul(bc[:], ones_r[:], st2_sb[:], start=True, stop=True)
            nc.vector.tensor_copy(totals[:], bc[:])
            nc.vector.tensor_tensor(s4[:], totals[:], inv_n[:], op=OP.mult)
            nc.vector.tensor_scalar(thr4[:], s4[:], EPS, ATANH05, op0=OP.add, op1=OP.mult)
            nc.vector.tensor_scalar(nthr4[:], thr4[:], -1.0, None, op0=OP.mult)
            # a4 = 0.5*ln((1+s)/(1-s)) = arctanh(s)
            num = sm.tile([128, 4], F32, tag="num")
            den = sm.tile([128, 4], F32, tag="den")
            rat = sm.tile([128, 4], F32, tag="rat")
            nc.vector.tensor_scalar(num[:], s4[:], 1.0, None, op0=OP.add)
            nc.vector.tensor_scalar(den[:], s4[:], -1.0, 1.0, op0=OP.mult, op1=OP.add)
            nc.vector.reciprocal(rat[:], den[:])
            nc.vector.tensor_tensor(rat[:], rat[:], num[:], op=OP.mult)
            lnr = sm.tile([128, 4], F32, tag="lnr")
            nc.scalar.activation(lnr[:], rat[:], AF.Ln)
            nc.vector.tensor_scalar(a4[:], lnr[:], 0.5, None, op0=OP.mult)
            nc.vector.tensor_scalar(aq128[:], a4[:, 0:1], 1.0 / 128.0, None, op0=OP.mult)

            # ternary: tern(w) = (w >= thr) - (w < -thr), exact in bf16
            with tc.tile_pool(name="tern", bufs=2, side="right") as tp_:
                def ternary(src_ap, w, thr_t, nthr_t, out_ap, flip=0):
                    a_ = tp_.tile([128, w], BF16, tag="ta")
                    b_ = tp_.tile([128, w], BF16, tag="tb")
                    ea, eb = ((nc.vector, nc.gpsimd) if flip == 0
                              else (nc.gpsimd, nc.vector))
                    ea.tensor_scalar(a_[:, :w], src_ap, thr_t, None, op0=OP.is_ge)
                    eb.tensor_scalar(b_[:, :w], src_ap, nthr_t, -1.0,
                                     op0=OP.is_lt, op1=OP.mult)
                    nc.vector.tensor_tensor(out_ap, a_[:, :w], b_[:, :w], op=OP.add)

                for j in range(NB):
                    tq = tp_.tile([128, KVD], BF16, tag="tq")
                    ternary(wqf[:, j, :], KVD, thr4[:, 0:1], nthr4[:, 0:1],
                            tq[:, :], flip=j % 2)
                    e1 = tp_.tile([128, HD], BF16, tag="e1")
                    e2 = tp_.tile([128, HD], BF16, tag="e2")
                    nc.vector.tensor_tensor(e1[:], tq[:, 0:HD], tq[:, HD:2 * HD],
                                            op=OP.add)
                    nc.gpsimd.tensor_tensor(e2[:], tq[:, 2 * HD:3 * HD],
                                            tq[:, 3 * HD:4 * HD], op=OP.add)
                    nc.vector.tensor_tensor(wqkv_i[j][:, 0:HD], e1[:], e2[:], op=OP.add)
                    ternary(wkvf[:, j, 0:HD], HD, thr4[:, 1:2], nthr4[:, 1:2],
                            wqkv_i[j][:, HD:2 * HD], flip=j % 2)
                    ternary(wkvf[:, j, HD:2 * HD], HD, thr4[:, 2:3], nthr4[:, 2:3],
                            wqkv_i[j][:, 2 * HD:3 * HD], flip=(j + 1) % 2)

        # ---------- x phase fused with QKV (per-2-block waves) ----------
        sq_scr = xph.tile([128, D], BF16, tag="sqscr")
        hh = HD // 2
        xqTb_of = {}
        ropes_of = {}

        def emit_quant_T(i, xb):
            xm = xph.tile([128, D], F32, tag="xm", bufs=2)
            if i % 3 == 0:
                nc.scalar.activation(xm[:], xb[:], AF.Copy,
                                     scale=smul_all[:, i:i + 1], bias=MAGIC)
            else:
                nc.gpsimd.tensor_scalar(xm[:], xb[:],
                                        smul_all[:, i:i + 1], MAGIC,
                                        op0=OP.mult, op1=OP.add)
            xqTb = xph.tile([128, NB, 128], BF16, tag="xqTb", bufs=4)
            for jj in range(4):
                tp4 = pstp.tile([128, 512], F32, tag="tp")
                for t4 in range(4):
                    nc.tensor.transpose(
                        tp4[:, t4 * 128:(t4 + 1) * 128],
                        xm[:, (4 * jj + t4) * 128:(4 * jj + t4 + 1) * 128],
                        idf[:])
                dst = xqTb[:, 4 * jj:4 * jj + 4, :]
                if (i * 4 + jj) % 2 == 0:
                    nc.scalar.activation(dst, tp4[:], AF.Copy, bias=-MAGIC)
                else:
                    nc.vector.tensor_scalar(dst, tp4[:], -MAGIC, None, op0=OP.add)
            xqTb_of[i] = xqTb

        dq_done = set()

        def emit_qkvmm(i):
            wv0 = i // 2
            if wv0 not in dq_done:
                dq_done.add(wv0)
                sl = slice(wv0 * 2, wv0 * 2 + 2)
                nc.vector.tensor_scalar(dq_all[:, sl], deq_all[:, sl],
                                        aq128[:], None, op0=OP.mult)
                nc.vector.tensor_scalar(dk_all[:, sl], deq_all[:, sl],
                                        a4[:, 1:2], None, op0=OP.mult)
                nc.vector.tensor_scalar(dv_all[:, sl], deq_all[:, sl],
                                        a4[:, 2:3], None, op0=OP.mult)
            xqTb = xqTb_of.pop(i)
            pq = psmm.tile([128, 512], F32, tag="mm")
            for j in range(NB):
                nc.tensor.matmul(pq[:, 0:3 * HD], xqTb[:, j, :],
                                 wqkv_i[j][:], start=(j == 0), stop=(j == NB - 1))
            qn = qkv.tile([128, HD], F32, tag="qn", bufs=3)
            kn = qkv.tile([128, HD], F32, tag="kn", bufs=3)
            nc.vector.tensor_scalar(qn[:], pq[:, 0:HD],
                                    dq_all[:, i:i + 1], None, op0=OP.mult)
            nc.scalar.activation(kn[:], pq[:, HD:2 * HD], AF.Copy,
                                 scale=dk_all[:, i:i + 1])
            nc.scalar.activation(v_aug[:, i, 0:HD], pq[:, 2 * HD:3 * HD],
                                 AF.Copy, scale=dv_all[:, i:i + 1])
            ci = cos_all[:, i, :]
            si = sin_all[:, i, :]
            qr = qkv.tile([128, HD], F32, tag="qr", bufs=3)
            kr = qkv.tile([128, HD], F32, tag="kr", bufs=3)
            for src_, dst_, eng, tg in ((qn, qr, nc.gpsimd, "q"),
                                        (kn, kr, nc.gpsimd, "k")):
                t1 = qkv.tile([128, hh], F32, tag=f"rt1{tg}", bufs=2)
                t2 = qkv.tile([128, hh], F32, tag=f"rt2{tg}", bufs=2)
                eng.tensor_tensor(t1[:], src_[:, 0:hh], ci, op=OP.mult)
                eng.tensor_tensor(t2[:], src_[:, hh:HD], si, op=OP.mult)
                eng.tensor_tensor(dst_[:, 0:hh], t1[:], t2[:], op=OP.subtract)
                t3 = qkv.tile([128, hh], F32, tag=f"rt3{tg}", bufs=2)
                t4_ = qkv.tile([128, hh], F32, tag=f"rt4{tg}", bufs=2)
                eng.tensor_tensor(t3[:], src_[:, 0:hh], si, op=OP.mult)
                eng.tensor_tensor(t4_[:], src_[:, hh:HD], ci, op=OP.mult)
                eng.tensor_tensor(dst_[:, hh:HD], t3[:], t4_[:], op=OP.add)
            ropes_of[i] = (qr, kr)

        def emit_qkT(i):
            qr, kr = ropes_of.pop(i)
            tqk = pstp.tile([128, 512], F32, tag="tp")
            nc.tensor.transpose(tqk[:, 0:128], qr[:], idf[:])
            nc.tensor.transpose(tqk[:, 128:256], kr[:], idf[:])
            nc.scalar.activation(qT[:, i * 128:(i + 1) * 128], tqk[:, 0:128], AF.Copy)
            nc.vector.tensor_copy(kT[:, i * 128:(i + 1) * 128], tqk[:, 128:256])
            if causal and i % 4 == 3:
                emit_attn_group(i // 4)
            elif not causal and i == NB - 1:
                for qg_ in range(4):
                    emit_attn_group(qg_)

        def emit_wave(wv_, drain):
            i0 = wv_ * 2
            xw = []
            for u in range(2):
                i = i0 + u
                xb = xph.tile([128, D], F32, tag=f"xb{i % 3}", name="xb")
                xw.append(xb)
                nc.sync.dma_start(xb[:], x_d.ap()[i * 128:(i + 1) * 128, :])
                nc.scalar.activation(sq_scr[:], xb[:], AF.Square,
                                     accum_out=ssq_all[:, i:i + 1])
                nc.vector.tensor_reduce(mx_all[:, i:i + 1], xb[:],
                                        axis=AX.X, op=OP.max,
                                        apply_absolute_value=True)
            sl = slice(i0, i0 + 2)
            mean2 = xph.tile([128, 2], F32, tag="mean2", bufs=2)
            sd2 = xph.tile([128, 2], F32, tag="sd2", bufs=2)
            r2 = xph.tile([128, 2], F32, tag="r2", bufs=2)
            m2 = xph.tile([128, 2], F32, tag="m2", bufs=2)
            im2 = xph.tile([128, 2], F32, tag="im2", bufs=2)
            nc.vector.tensor_scalar(mean2[:], ssq_all[:, sl], 1.0 / D, EPS,
                                    op0=OP.mult, op1=OP.add)
            nc.scalar.activation(sd2[:], mean2[:], AF.Sqrt)
            nc.vector.reciprocal(r2[:], sd2[:])
            nc.vector.tensor_tensor(m2[:], r2[:], mx_all[:, sl], op=OP.mult)
            nc.vector.tensor_scalar(m2[:], m2[:], 1e-4, None, op0=OP.max)
            nc.vector.tensor_scalar(deq_all[:, sl], m2[:], 1.0 / 127.0,
                                    None, op0=OP.mult)
            nc.vector.reciprocal(im2[:], m2[:])
            nc.vector.tensor_scalar(im2[:], im2[:], 127.0, None, op0=OP.mult)
            nc.vector.tensor_tensor(smul_all[:, sl], r2[:], im2[:], op=OP.mult)
            for u in range(2):
                i = i0 + u
                emit_quant_T(i, xw[u])
                if drain:
                    while len(xqTb_of) > 1:
                        emit_qkvmm(min(xqTb_of))
                    while len(ropes_of) > 1:
                        emit_qkT(min(ropes_of))

        # ---------- attention machinery (emitted interleaved with X/QKV) -----
        # S^T[key,query] via matmul(lhsT=kT, rhs=qT); exp fused into PSUM
        # eviction; Z via ones-column in v_aug. Steps of query-group qg are
        # emitted as soon as qkT(4*qg+3) exists. PE stream pipelined 2 deep.
        from collections import deque
        accs_by_qg = {}
        pendq = deque()

        def emit_pv(qg, kb, pt, q0b):
            accs = accs_by_qg[qg]
            for i_ in range(q0b, 4 * qg + 4):
                t = i_ - 4 * qg
                off = (i_ - q0b) * 128
                done = (kb == i_) if causal else (kb == NB - 1)
                nc.tensor.matmul(accs[t], pt[:, off:off + 128],
                                 v_aug[:, kb, 0:HD + 1],
                                 start=(kb == 0), stop=done)
                if done:
                    rz = attn.tile([128, 1], F32, tag="rz", bufs=2)
                    nc.vector.reciprocal(rz[:], accs[t][:, HD:HD + 1])
                    ob = attn.tile([128, HD], BF16, tag="ob", bufs=2)
                    nc.scalar.activation(ob[:], accs[t][:, 0:HD],
                                         AF.Copy, scale=rz[:])
                    nc.sync.dma_start(cc_in[qg, t * 128:(t + 1) * 128, :], ob[:])

        def emit_attn_step(qg, kb):
            if kb == 0:
                accs_by_qg[qg] = [
                    pacc.tile([128, 132], F32, tag=f"acc{t}", name=f"acc{t}")[:, 0:HD + 1]
                    for t in range(4)]
            q0b = max(4 * qg, kb) if causal else 4 * qg
            w = (4 * qg + 4 - q0b) * 128
            ps = psmm.tile([128, 512], F32, tag="mm")
            nc.tensor.matmul(ps[:, 0:w], kT[:, kb * 128:(kb + 1) * 128],
                             qT[:, q0b * 128:q0b * 128 + w], start=True, stop=True)
            if causal and kb >= 4 * qg:
                nc.vector.tensor_tensor(ps[:, 0:128], ps[:, 0:128], cmT[:], op=OP.add)
            pt = attn.tile([128, 512], BF16, tag="pt", bufs=4)
            nc.scalar.activation(pt[:, 0:w], ps[:, 0:w], AF.Exp)
            pendq.append((qg, kb, pt, q0b))
            if len(pendq) > 2:
                emit_pv(*pendq.popleft())

        def emit_attn_group(qg):
            nkb = (4 * qg + 4) if causal else NB
            for kb in range(nkb):
                emit_attn_step(qg, kb)

        emit_wave(0, False)
        emit_wave(1, False)
        emit_ar_chain()
        emit_ternary()
        nc.sync.dma_start(cos_all[:], cos_d.ap().rearrange("(i p) f -> p i f", p=128))
        nc.sync.dma_start(sin_all[:], sin_d.ap().rearrange("(i p) f -> p i f", p=128))
        for wv_ in range(2, NB // 2):
            emit_wave(wv_, True)
        while xqTb_of:
            emit_qkvmm(min(xqTb_of))
        while ropes_of:
            emit_qkT(min(ropes_of))
        while pendq:
            emit_pv(*pendq.popleft())

        # ---------- w_o: one resident pass (local sum, then ternary) ---------
        # w_o is replicated on all cores -> local mean, no collective.
        with tc.tile_pool(name="wo2", bufs=1, side="right") as wo2:
            woss = []
            for c in range(4):
                wos = wo2.tile([128, D], F32, tag=f"wos{c}", name="wos")
                woss.append(wos)
                nc.sync.dma_start(wos[:], wo_d.ap()[c * 128:(c + 1) * 128, :])
                nc.vector.tensor_reduce(wop[:, c:c + 1], wos[:], axis=AX.X,
                                        op=OP.add, apply_absolute_value=True)
            wot = sm.tile([128, 1], F32, tag="wot")
            nc.vector.tensor_reduce(wot[:], wop[:], axis=AX.X, op=OP.add)
            pso = psmm.tile([1, 4], F32, tag="mm")
            nc.tensor.matmul(pso[:, 0:1], ones_c[:], wot[:], start=True, stop=True)
            nc.vector.tensor_copy(so_row[:], pso[:, 0:1])
            bco = psmm.tile([128, 4], F32, tag="mm")
            nc.tensor.matmul(bco[:, 0:1], ones_r[:], so_row[:], start=True, stop=True)
            nc.vector.tensor_scalar(so_t[:], bco[:, 0:1], 1.0 / (KVD * D), None,
                                    op0=OP.mult)
            nc.vector.tensor_scalar(thr_o[:], so_t[:], EPS, ATANH05,
                                    op0=OP.add, op1=OP.mult)
            nc.vector.tensor_scalar(nthr_o[:], thr_o[:], -1.0, None, op0=OP.mult)
            numo = sm.tile([128, 1], F32, tag="numo")
            deno = sm.tile([128, 1], F32, tag="deno")
            rato = sm.tile([128, 1], F32, tag="rato")
            nc.vector.tensor_scalar(numo[:], so_t[:], 1.0, None, op0=OP.add)
            nc.vector.tensor_scalar(deno[:], so_t[:], -1.0, 1.0, op0=OP.mult, op1=OP.add)
            nc.vector.reciprocal(rato[:], deno[:])
            nc.vector.tensor_tensor(rato[:], rato[:], numo[:], op=OP.mult)
            lno = sm.tile([128, 1], F32, tag="lno")
            nc.scalar.activation(lno[:], rato[:], AF.Ln)
            nc.vector.tensor_scalar(a_o[:], lno[:], 0.5, None, op0=OP.mult)
            for c in range(4):
                for hf in range(2):
                    slc = slice(hf * 1024, (hf + 1) * 1024)
                    a_ = wo2.tile([128, 1024], BF16, tag="toa", bufs=2)
                    b_ = wo2.tile([128, 1024], BF16, tag="tob", bufs=2)
                    nc.vector.tensor_scalar(a_[:], woss[c][:, slc], thr_o[:], None,
                                            op0=OP.is_ge)
                    nc.vector.tensor_scalar(b_[:], woss[c][:, slc], nthr_o[:], -1.0,
                                            op0=OP.is_lt, op1=OP.mult)
                    nc.vector.tensor_tensor(wo_i[c][:, slc], a_[:], b_[:], op=OP.add)

        # ---------- exchange: quad AllToAll (within batch group) -------------
        if local_cc:
            nc.sync.dma_start(cc_out.ap(), cc_in.ap())
        else:
            nc.gpsimd.collective_compute(
                "AllToAll", OP.bypass,
                replica_groups=[[0, 1, 2, 3], [4, 5, 6, 7]],
                ins=[cc_in.ap().opt()], outs=[cc_out.ap().opt()])

        # ---------- output projection ----------
        xoT = outp.tile([128, 4, SQ], BF16, tag="xoT")
        osc = outp.tile([128, KVD], BF16, tag="osc")
        xo4s = [outp.tile([128, 4, HD], BF16, tag=f"xo{tb}", name=f"xo{tb}")
                for tb in range(4)]
        sso = outp.tile([128, 4], F32, tag="sso")
        mxo = outp.tile([128, 4], F32, tag="mxo")
        for tb in range(4):
            nc.sync.dma_start(
                xo4s[tb][:],
                cc_out.ap()[:, tb * 128:(tb + 1) * 128, :].rearrange("j p d -> p j d"))
            nc.scalar.activation(osc[:], xo4s[tb][:], AF.Square,
                                 accum_out=sso[:, tb:tb + 1])
            nc.vector.tensor_reduce(mxo[:, tb:tb + 1], xo4s[tb][:],
                                    axis=AX.XY, op=OP.max, apply_absolute_value=True)
        meano = outp.tile([128, 4], F32, tag="meano")
        sdo = outp.tile([128, 4], F32, tag="sdo")
        ro = outp.tile([128, 4], F32, tag="ro")
        mo = outp.tile([128, 4], F32, tag="mo")
        imo = outp.tile([128, 4], F32, tag="imo")
        smul_o = outp.tile([128, 4], F32, tag="smul_o")
        dqy_o = outp.tile([128, 4], F32, tag="dqy_o")
        nc.vector.tensor_scalar(meano[:], sso[:], 1.0 / KVD, EPS, op0=OP.mult, op1=OP.add)
        nc.scalar.activation(sdo[:], meano[:], AF.Sqrt)
        nc.vector.reciprocal(ro[:], sdo[:])
        nc.vector.tensor_tensor(mo[:], ro[:], mxo[:], op=OP.mult)
        nc.vector.tensor_scalar(mo[:], mo[:], 1e-4, None, op0=OP.max)
        nc.vector.tensor_scalar(dqy_o[:], mo[:], 1.0 / 127.0, None, op0=OP.mult)
        nc.vector.tensor_scalar(dqy_o[:], dqy_o[:], a_o[:], None, op0=OP.mult)
        nc.vector.reciprocal(imo[:], mo[:])
        nc.vector.tensor_scalar(imo[:], imo[:], 127.0, None, op0=OP.mult)
        nc.vector.tensor_tensor(smul_o[:], ro[:], imo[:], op=OP.mult)

        def emit_oT(tb):
            xmo = outp.tile([128, KVD], F32, tag="xmo", bufs=2)
            nc.scalar.activation(xmo[:], xo4s[tb][:], AF.Copy,
                                 scale=smul_o[:, tb:tb + 1], bias=MAGIC)
            tpo = pstp.tile([128, 512], F32, tag="tp")
            for jc in range(4):
                nc.tensor.transpose(tpo[:, jc * 128:(jc + 1) * 128],
                                    xmo[:, jc * 128:(jc + 1) * 128], idf[:])
            dst = xoT[:, 0:4, tb * 128:(tb + 1) * 128]
            if tb % 2 == 0:
                nc.vector.tensor_scalar(dst, tpo[:], -MAGIC, None, op0=OP.add)
            else:
                nc.scalar.activation(dst, tpo[:], AF.Copy, bias=-MAGIC)

        def emit_omm(tb):
            y_sb = outp.tile([128, D], BF16, tag="ysb", bufs=2)
            for oc in range(4):
                py = psmm.tile([128, 512], F32, tag="mm")
                for jc in range(4):
                    nc.tensor.matmul(py[:], xoT[:, jc, tb * 128:(tb + 1) * 128],
                                     wo_i[jc][:, oc * 512:(oc + 1) * 512],
                                     start=(jc == 0), stop=(jc == 3))
                nc.scalar.activation(y_sb[:, oc * 512:(oc + 1) * 512], py[:],
                                     AF.Copy, scale=dqy_o[:, tb:tb + 1])
            nc.sync.dma_start(y_d[tb * 128:(tb + 1) * 128, :], y_sb[:])

        for tb in range(4):
            emit_oT(tb)
            if tb >= 1:
                emit_omm(tb - 1)
        emit_omm(3)
    nc.compile()
    return nc


def _rope_perm():
    p = np.empty(HD, np.int64)
    p[:HD // 2] = np.arange(0, HD, 2)
    p[HD // 2:] = np.arange(1, HD, 2)
    return p


def _prep_inputs(inputs):
    x = np.ascontiguousarray(np.asarray(inputs["x"], np.float32))
    w_q = np.asarray(inputs["w_q"], np.float32)
    w_k = np.asarray(inputs["w_k"], np.float32)
    w_v = np.asarray(inputs["w_v"], np.float32)
    w_o = np.asarray(inputs["w_o"], np.float32)
    cos = np.ascontiguousarray(np.asarray(inputs["freq_cos"], np.float32))
    sin = np.ascontiguousarray(np.asarray(inputs["freq_sin"], np.float32))
    perm = _rope_perm()
    woT = np.ascontiguousarray(w_o.T)                      # [KVD, D]
    in_maps = []
    for r in range(8):
        b, kh = r // 4, r % 4
        heads = [g * KH + kh for g in range(4)]
        wq_sel = w_q.reshape(H, HD, D)[heads][:, perm, :]  # [4,128,D]
        wqT = np.ascontiguousarray(wq_sel.reshape(4 * HD, D).T)   # [D, 512]
        wkT = np.ascontiguousarray(w_k[kh * HD:(kh + 1) * HD][perm].T)  # [D,128]
        wvT = np.ascontiguousarray(w_v[kh * HD:(kh + 1) * HD].T)        # [D,128]
        in_maps.append({
            "x": x[b], "wq": wqT, "wk": wkT, "wv": wvT, "wo": woT,
            "cos": cos, "sin": sin,
        })
    return in_maps


def _gains_trivial(inputs):
    return all(np.all(np.asarray(inputs[g]) == 1.0)
               for g in ("g_q", "g_k", "g_v", "g_o"))


def _numpy_fallback(inputs):
    """Faithful numpy reimplementation (slow); used only for unexpected configs."""
    x = np.asarray(inputs["x"], np.float32)
    cos, sin = (np.asarray(inputs[k], np.float32) for k in ("freq_cos", "freq_sin"))
    causal = int(np.asarray(inputs["causal"]))

    def rms(t, g):
        n = t * (1.0 / np.sqrt(np.mean(t * t, -1, keepdims=True, dtype=np.float32) + EPS))
        return (g * n).astype(np.float32)

    def actq(t):
        scale = 127.0 / np.clip(np.max(np.abs(t), -1, keepdims=True), 1e-4, None)
        q = np.round(t * scale)
        return np.clip(q, -128, 127) / scale

    def ternq(w):
        s = np.mean(np.abs(w), dtype=np.float32)
        return np.round(np.tanh(w / (s + EPS))) * np.arctanh(s)

    def lin(t, w, g):
        return actq(rms(t, g)).astype(np.float32) @ ternq(np.asarray(w, np.float32)).T

    Bb, Ss, Dd = x.shape
    q = lin(x, inputs["w_q"], np.asarray(inputs["g_q"], np.float32)).reshape(Bb, Ss, H, HD)
    k = lin(x, inputs["w_k"], np.asarray(inputs["g_k"], np.float32)).reshape(Bb, Ss, KH, HD)
    v = lin(x, inputs["w_v"], np.asarray(inputs["g_v"], np.float32)).reshape(Bb, Ss, KH, HD)

    def rope(t):
        t2 = t.reshape(*t.shape[:-1], -1, 2)
        c = cos[None, :, None, :]
        s_ = sin[None, :, None, :]
        o0 = t2[..., 0] * c - t2[..., 1] * s_
        o1 = t2[..., 0] * s_ + t2[..., 1] * c
        return np.stack([o0, o1], -1).reshape(t.shape).astype(np.float32)

    q, k = rope(q), rope(k)
    scale = np.float32(HD ** 0.5)
    q = q.transpose(0, 2, 1, 3) / scale
    k = k.transpose(0, 2, 1, 3)
    v = v.transpose(0, 2, 1, 3)
    qg = q.reshape(Bb, 4, KH, Ss, HD).sum(1)
    sc = np.einsum("bhnd,bhsd->bhns", qg, k).astype(np.float32)
    if causal:
        mask = np.tril(np.ones((Ss, Ss), bool))
        sc = np.where(mask[None, None], sc, np.float32(np.finfo(np.float32).min))
    sc = sc / scale
    sc = sc - sc.max(-1, keepdims=True)
    p = np.exp(sc)
    p /= p.sum(-1, keepdims=True)
    out = np.einsum("bhns,bhsd->bnhd", p, v).reshape(Bb, Ss, KVD)
    return lin(out, inputs["w_o"], np.asarray(inputs["g_o"], np.float32))


def kernel(**inputs):
    x = np.asarray(inputs["x"])
    if x.shape != (B, S, D) or not _gains_trivial(inputs):
        return _numpy_fallback(inputs)
    causal = bool(int(np.asarray(inputs["causal"])))
    key = ("bitattn", causal)
    if key not in _cache:
        _cache[key] = build(causal)
    nc = _cache[key]
    in_maps = _prep_inputs(inputs)
    res = run_bass_kernel_spmd(nc, in_maps, core_ids=list(range(8)))
    y = np.empty((B, S, D), np.float32)
    for r in range(8):
        b, qq = r // 4, r % 4
        y[b, qq * SQ:(qq + 1) * SQ, :] = np.asarray(res.results[r]["y"],
                                                    dtype=np.float32)
    return y


if __name__ == "__main__":
    data = np.load("/tmp/inputs.npz")
    inputs = {k: data[k] for k in data.files}
    out = kernel(**inputs)
    exp = np.load("/tmp/expected.npy")
    err = np.linalg.norm(out - exp) / np.linalg.norm(exp)
    print("Relative error:", err)
